# revision 17
# baseline (speedup 1.0000x reference)
"""Block-diagonal GRU cell for Trainium2, data-parallel over 8 NeuronCores.

Math (per batch row b, block j of 8, block size 256):
    wx  = x @ W_ir.T + b_ir_lin + b_ir          # [B, 6144], gates r|z|n global-chunked
    wh  = hb_j @ W_h[j].T + b_hr_j              # per block, local r|z|n chunks of 256
    r   = sigmoid(wxr + whr)
    z   = sigmoid(wxz + whz)
    n   = tanh(wxn + r * whn)
    h'  = (1-z)*hb + z*n

Device strategy (per core, batch-sharded 1024 rows):
  - The kernel is PE-bound (fp16 matmul floor ~205us vs ~100us memory floor),
    so the error-tolerant gate paths run as fp8e4 DoubleRow matmuls (2x PE
    throughput, one instruction per TWO 128-deep k-subtiles):
      * r/z x-projection and h-projection: fp8 operands
      * whn (the h-side n term, damped by r and tanh'): fp8
      * wxn stays fp16 -- its error hits tanh at derivative ~1 and dominates;
        all-fp8 sims at 2.1e-2 rel err (gate 2e-2) vs 1.24e-2 for this split.
  - fp8 weights are pre-scaled by 64 on the host so W*64 ~ N(0,1.3) clears
    the e4m3 min-normal (2^-6); x/h quantize unscaled (sigma=1). The n-path
    fp16 weights carry the same *64 so both PSUM banks are uniformly scaled:
    the r/z sigmoid uses scale=1/64 and the tanh-sigmoid scale=2/64.
  - Blocks are processed in PAIRS (jp = j/2) so every epilogue op is 512
    wide: per-instruction overhead (~150-250ns on ACT/DVE) was ~30% of both
    engines' time at 256-wide. PSUM: A0,A1 (r|z sums per block, 4-bank pool)
    + Bx [wxn j0|j1] + Bh [whn j0|whn j1] (2-bank pools) = 8 banks.
  - The epilogue runs in fp16 so every non-PSUM DVE op qualifies for the
    DVE 2x_1p perf mode (all operands 2-byte packed); the final op is a
    tensor_scalar (1-source) which gets 2x_2p with an fp32 destination.
    With hb1h = fp16((h+1)/2) prepared on the host:
        tn = sigmoid(2*a_n)               n = 2*tn - 1
        d2 = tn - hb1h                    (= (n - hb)/2)
        t5 = z * d2
        u  = t5 + hb1h                    (= (h' + 1)/2)
        out= 2*u - 1                      tensor_scalar, fp32 store
    This keeps tanh on the single sigmoid ACT table (no table reloads).
  - Weights are host-reordered so every DMA is wide and contiguous and every
    matmul rhs is a plain slice:
        wrz8 [IN, 8*512]  cols = block j -> [W_ir r-rows j | W_ir z-rows j]^T
        wn16 [IN, 8*256], whrz8 [BS, 8*512], whn8 [BS, 8*256] from W_h
  - x^T (fp16 + fp8 copies) and h^T (fp8) are host-transposed (stationary
    operands need partition = contraction dim).
  - Everything is SBUF-resident (~172KB/partition). Descriptor pushes cost
    ~800ns on the issuing engine, so the head uses few, wide DMAs split
    across the SP ring and the (store-idle at head) ACT ring, ordered so
    pair 0's matmul operands land first; later weight pairs stream in
    whole-pair pushes well ahead of use.
  - Steady-state loads ride the SP HWDGE ring, stores the ACT ring.
"""

import sys

if "/opt/trn_rl_repo" not in sys.path:
    sys.path.insert(0, "/opt/trn_rl_repo")

import numpy as np

B, IN, H, NB = 8192, 1024, 2048, 8
BS = H // NB  # 256
NCORES = 8
BC = B // NCORES  # 1024 rows per core
P = 128
WSCALE = 64.0  # fp8 weight pre-scale; folded back via activation scales

_BUILD_CACHE = {}


def build_nc(bc=BC, has_bias=False):
    """Build the Bass program for one core (SPMD: same program on all 8)."""
    key = (bc, has_bias)
    if key in _BUILD_CACHE:
        return _BUILD_CACHE[key]

    from contextlib import ExitStack

    import concourse.bacc as bacc
    import concourse.mybir as mybir
    import concourse.tile as tile

    f8 = mybir.dt.float8e4
    f16 = mybir.dt.float16
    f32 = mybir.dt.float32
    SIG = mybir.ActivationFunctionType.Sigmoid
    DR = mybir.MatmulPerfMode.DoubleRow
    MULT = mybir.AluOpType.mult
    ADD = mybir.AluOpType.add

    K1 = IN // P  # 8 contraction chunks for the x projection
    K2 = BS // P  # 2 contraction chunks per block for the h projection
    MT = bc // P  # m-tiles (128 batch rows each)
    NP = NB // 2  # block pairs

    # Bacc (not plain Bass): its compile() runs move_matmul_waits_to_ldweights
    # + generate_event_semaphores, which split multi-sem waits down to the
    # 1-wait-per-instruction TRN2 ISA budget.
    nc = bacc.Bacc(target_bir_lowering=False)

    xt = nc.dram_tensor("xt", [IN, bc], f16, kind="ExternalInput").ap()
    x8 = nc.dram_tensor("x8", [IN, bc], f8, kind="ExternalInput").ap()
    # ht8 is host-swizzled to [128, 16, bc] (partition-major) so its two
    # head DMAs coalesce to one fat contiguous descriptor per partition;
    # the naive [H, bc] layout cost a 16us descriptor-generation push.
    ht8 = nc.dram_tensor("ht8", [P, 2 * K2 * (NB // 2), bc], f8,
                         kind="ExternalInput").ap()
    hb1h = nc.dram_tensor("hb1h", [bc, H], f16, kind="ExternalInput").ap()
    wrz8 = nc.dram_tensor("wrz8", [IN, NB * 2 * BS], f8, kind="ExternalInput").ap()
    wn = nc.dram_tensor("wn", [IN, NB * BS], f16, kind="ExternalInput").ap()
    whrz8 = nc.dram_tensor("whrz8", [BS, NB * 2 * BS], f8, kind="ExternalInput").ap()
    whn8 = nc.dram_tensor("whn8", [BS, NB * BS], f8, kind="ExternalInput").ap()
    if has_bias:
        brz_d = nc.dram_tensor("brz", [1, NB * 2 * BS], f32, kind="ExternalInput").ap()
        bxn_d = nc.dram_tensor("bxn", [1, NB * BS], f32, kind="ExternalInput").ap()
        bhn_d = nc.dram_tensor("bhn", [1, NB * BS], f32, kind="ExternalInput").ap()
    out = nc.dram_tensor("out", [bc, H], f32, kind="ExternalOutput").ap()

    xt_r = xt.rearrange("(k p) b -> p k b", p=P)  # [128, K1, bc]
    x8_r = x8.rearrange("(k p) b -> p k b", p=P)  # [128, K1, bc]
    ht8_r = ht8  # already [128, 16, bc]
    wrz8_r = wrz8.rearrange("(k p) f -> p k f", p=P)
    wn_r = wn.rearrange("(k p) f -> p k f", p=P)
    whrz8_r = whrz8.rearrange("(k p) f -> p k f", p=P)
    whn8_r = whn8.rearrange("(k p) f -> p k f", p=P)

    hb1h_r = hb1h.rearrange("(m p) h -> p m h", p=P)  # [128, MT, H]

    with tile.TileContext(nc) as tc, ExitStack() as ctx:
        wpool = ctx.enter_context(tc.tile_pool(name="wres", bufs=1))
        psA = ctx.enter_context(tc.tile_pool(name="psA", bufs=4, space="PSUM"))
        psBx = ctx.enter_context(tc.tile_pool(name="psBx", bufs=2, space="PSUM"))
        psBh = ctx.enter_context(tc.tile_pool(name="psBh", bufs=2, space="PSUM"))
        epool = ctx.enter_context(tc.tile_pool(name="epi", bufs=3))

        # ---- resident tiles (everything is resident: ~172KB/partition) ----
        # Weight DMAs are pair-column-major and the compute loop is jp-outer /
        # m-inner: one pair's weight columns (~2.4MB) unlock ~35us of
        # matmuls across all m-tiles, so the PE never starves on the preload.
        xt_sb = wpool.tile([P, K1, bc], f16, tag="xt_sb")
        x8_sb = wpool.tile([P, K1, bc], f8, tag="x8_sb")
        ht_sb = wpool.tile([P, 2 * K2 * NP, bc], f8, tag="ht_sb")
        hb_sb = wpool.tile([P, MT, H], f16, tag="hb_sb")
        wrz8_sb = wpool.tile([P, K1, NB * 2 * BS], f8, tag="wrz8_sb")
        wn_sb = wpool.tile([P, K1, NB * BS], f16, tag="wn_sb")
        whrz8_sb = wpool.tile([P, K2, NB * 2 * BS], f8, tag="whrz8_sb")
        whn8_sb = wpool.tile([P, K2, NB * BS], f8, tag="whn8_sb")

        def load_w_pair(jp):
            # one wide push per weight tensor per pair (on the sync ring)
            rzsl = slice(2 * jp * 2 * BS, (2 * jp + 2) * 2 * BS)
            nsl = slice(2 * jp * BS, (2 * jp + 2) * BS)
            nc.sync.dma_start(wrz8_sb[:, :, rzsl], wrz8_r[:, :, rzsl])
            nc.sync.dma_start(wn_sb[:, :, nsl], wn_r[:, :, nsl])
            nc.sync.dma_start(whrz8_sb[:, :, rzsl], whrz8_r[:, :, rzsl])
            nc.sync.dma_start(whn8_sb[:, :, nsl], whn8_r[:, :, nsl])

        # prewarm the ACT sigmoid table (~2.7us ACT_TABLE_LOAD) at t~0 so
        # the first real epilogue doesn't pay it inline right when the PE's
        # PSUM bank rotation depends on that sigmoid releasing bank A
        ws = wpool.tile([P, 1], f32, tag="ws")
        nc.vector.memset(ws[:], 0.0)
        nc.scalar.activation(ws[:], ws[:], SIG)

        # Head loads. Descriptor pushes cost ~800ns each on the issuing
        # engine, so the head uses FEW, WIDE DMAs and splits them across the
        # sync ring and the (otherwise idle until the first store) ACT ring,
        # ordered so the first matmuls' operands land first.
        j0rz = slice(0, 2 * BS)
        # sync ring: fp8 x-projection operands for pair 0 (smallest first so
        # the first DoubleRow fires ~9us in), then j1, then the bulk
        nc.sync.dma_start(x8_sb[:, 0:4, 0:P], x8_r[:, 0:4, 0:P])
        nc.sync.dma_start(wrz8_sb[:, 0:4, j0rz], wrz8_r[:, 0:4, j0rz])
        nc.sync.dma_start(x8_sb[:, 4:K1, 0:P], x8_r[:, 4:K1, 0:P])
        nc.sync.dma_start(wrz8_sb[:, 4:K1, j0rz], wrz8_r[:, 4:K1, j0rz])
        nc.sync.dma_start(
            wrz8_sb[:, :, 2 * BS : 4 * BS], wrz8_r[:, :, 2 * BS : 4 * BS]
        )
        nc.sync.dma_start(ht_sb[:, 0:4, :], ht8_r[:, 0:4, :])
        nc.sync.dma_start(whrz8_sb[:, :, 0 : 4 * BS], whrz8_r[:, :, 0 : 4 * BS])
        nc.sync.dma_start(whn8_sb[:, :, 0 : 2 * BS], whn8_r[:, :, 0 : 2 * BS])
        nc.sync.dma_start(x8_sb[:, :, P:bc], x8_r[:, :, P:bc])
        nc.sync.dma_start(ht_sb[:, 4 : 2 * K2 * NP, :], ht8_r[:, 4 : 2 * K2 * NP, :])
        # ACT ring: fp16 n-path operands + the blend tensor, interleaved in
        # first-use order (m0's epilogue slice lands before the m1-7 bulk)
        nc.scalar.dma_start(xt_sb[:, :, 0:P], xt_r[:, :, 0:P])
        nc.scalar.dma_start(hb_sb[:, 0:2, 0 : 2 * BS], hb1h_r[:, 0:2, 0 : 2 * BS])
        nc.scalar.dma_start(wn_sb[:, :, 0 : 2 * BS], wn_r[:, :, 0 : 2 * BS])
        nc.scalar.dma_start(xt_sb[:, :, P:bc], xt_r[:, :, P:bc])
        nc.scalar.dma_start(hb_sb[:, 2:MT, 0 : 2 * BS], hb1h_r[:, 2:MT, 0 : 2 * BS])
        nc.scalar.dma_start(hb_sb[:, :, 2 * BS : H], hb1h_r[:, :, 2 * BS : H])
        load_w_pair(1)  # pair 1 isn't needed until ~35us
        if has_bias:
            ones_sb = wpool.tile([1, P], f32, tag="ones_sb")
            nc.vector.memset(ones_sb[:], 1.0)
            brz_sb = wpool.tile([1, NB * 2 * BS], f32, tag="brz_sb")
            bxn_sb = wpool.tile([1, NB * BS], f32, tag="bxn_sb")
            bhn_sb = wpool.tile([1, NB * BS], f32, tag="bhn_sb")
            nc.sync.dma_start(brz_sb[:], brz_d[:])
            nc.sync.dma_start(bxn_sb[:], bxn_d[:])
            nc.sync.dma_start(bhn_sb[:], bhn_d[:])

        for jp in range(NP):
            j0, j1 = 2 * jp, 2 * jp + 1
            jrz0 = slice(j0 * 2 * BS, (j0 + 1) * 2 * BS)
            jrz1 = slice(j1 * 2 * BS, (j1 + 1) * 2 * BS)
            jn0 = slice(j0 * BS, (j0 + 1) * BS)
            jn1 = slice(j1 * BS, (j1 + 1) * BS)
            jnp_ = slice(j0 * BS, (j1 + 1) * BS)  # pair's 512 n-columns
            josl = slice(j0 * BS, (j1 + 1) * BS)  # pair's 512 output columns
            for m in range(MT):
                msl = slice(m * P, (m + 1) * P)
                ht_mp = ht_sb[:, 4 * jp : 4 * jp + 4, msl]
                h_mp = hb_sb[:, m, josl]
                A0 = psA.tile([P, 2 * BS], f32, tag="A")
                A1 = psA.tile([P, 2 * BS], f32, tag="A")
                Bx = psBx.tile([P, 2 * BS], f32, tag="Bx")
                Bh = psBh.tile([P, 2, BS], f32, tag="Bh")
                # r/z paths: fp8 DoubleRow, two 128-deep k-subtiles per
                # instruction. Each A group start pending-zeroes its bank.
                for A, jrz, hk in ((A0, jrz0, slice(0, 2)), (A1, jrz1, slice(2, 4))):
                    for kk in range(K1 // 2):
                        ks = slice(2 * kk, 2 * kk + 2)
                        nc.tensor.matmul(
                            A[:, :], lhsT=x8_sb[:, ks, msl],
                            rhs=wrz8_sb[:, ks, jrz],
                            start=(kk == 0), stop=False, perf_mode=DR,
                        )
                    nc.tensor.matmul(
                        A[:, :], lhsT=ht_mp[:, hk, :], rhs=whrz8_sb[:, :, jrz],
                        start=False, stop=not has_bias, perf_mode=DR,
                    )
                # n path, h side: fp8 DoubleRow whn for both blocks into Bh
                nc.tensor.matmul(
                    Bh[:, 0, :], lhsT=ht_mp[:, 0:2, :], rhs=whn8_sb[:, :, jn0],
                    start=True, stop=False, perf_mode=DR,
                )
                nc.tensor.matmul(
                    Bh[:, 1, :], lhsT=ht_mp[:, 2:4, :], rhs=whn8_sb[:, :, jn1],
                    start=False, stop=not has_bias, perf_mode=DR,
                )
                # n path, x side: fp16 wxn chunks, 512 wide across the pair
                for k in range(K1):
                    nc.tensor.matmul(
                        Bx[:, :], lhsT=xt_sb[:, k, msl], rhs=wn_sb[:, k, jnp_],
                        start=(k == 0), stop=(k == K1 - 1) and not has_bias,
                    )
                if has_bias:
                    # rank-1 bias add: ones[K=1,128].T @ bias[K=1,N]
                    for A, jrz in ((A0, jrz0), (A1, jrz1)):
                        nc.tensor.matmul(
                            A[:, :], lhsT=ones_sb[:, :], rhs=brz_sb[:, jrz],
                            start=False, stop=True,
                        )
                    nc.tensor.matmul(
                        Bh[:, :, :], lhsT=ones_sb[:, :], rhs=bhn_sb[:, jnp_],
                        start=False, stop=True,
                    )
                    nc.tensor.matmul(
                        Bx[:, :], lhsT=ones_sb[:, :], rhs=bxn_sb[:, jnp_],
                        start=False, stop=True,
                    )

                # epilogue, all 512 wide; PSUM values carry the *64 weight
                # scale. fp16 intermediates turn on DVE 2x_1p for the
                # non-PSUM ops; rp/zp are strided views [2,256] (legal:
                # only the last dim must be packed).
                rz = epool.tile([P, 2, 2 * BS], f16, tag="rz")
                nc.scalar.activation(rz[:, 0, :], A0[:, :], SIG, scale=1.0 / WSCALE)
                nc.scalar.activation(rz[:, 1, :], A1[:, :], SIG, scale=1.0 / WSCALE)
                rp = rz[:, :, 0:BS]
                zp = rz[:, :, BS : 2 * BS]
                t3 = epool.tile([P, 2, BS], f16, tag="t3")
                nc.vector.tensor_mul(t3[:, :, :], rp, Bh[:, :, :])
                t4 = epool.tile([P, 2 * BS], f16, tag="t4")
                nc.vector.tensor_add(t4[:], Bx[:, :], t3[:, :, :])
                tn = epool.tile([P, 2 * BS], f16, tag="tn")
                nc.scalar.activation(tn[:], t4[:], SIG, scale=2.0 / WSCALE)
                d2 = epool.tile([P, 2 * BS], f16, tag="t4")
                nc.vector.tensor_sub(d2[:], tn[:], h_mp)
                t5 = epool.tile([P, 2 * BS], f16, tag="t5")
                nc.vector.tensor_mul(t5[:], zp, d2[:])
                u = epool.tile([P, 2 * BS], f16, tag="t3")
                nc.vector.tensor_add(u[:], t5[:], h_mp)
                oj = epool.tile([P, 2 * BS], f32, tag="oj")
                # the final affine runs on the otherwise-idle GPSIMD engine:
                # it frees ~0.5us/pair of DVE and shortens the tail backlog
                nc.gpsimd.tensor_scalar(oj[:], u[:], 2.0, -1.0, op0=MULT, op1=ADD)
                # stores ride the ACT HWDGE ring: the sync ring carries the
                # (prefetch-blocked) loads and would delay slot releases.
                # Except the final pair: by then the sync ring is idle while
                # ACT still has epilogue work queued ahead in its FIFO.
                if jp == NP - 1:
                    # final pair: alternate rings so the tail stores drain in
                    # parallel (sync is load-idle by now)
                    if m % 2 == 0:
                        nc.sync.dma_start(out[msl, josl], oj[:])
                    else:
                        nc.scalar.dma_start(out[msl, josl], oj[:])
                else:
                    nc.scalar.dma_start(out[msl, josl], oj[:])

            # prefetch the next pair's weights (pairs 0,1 loaded in the head)
            if 0 < jp < NP - 1:
                load_w_pair(jp + 1)

    nc.compile()
    _BUILD_CACHE[key] = nc
    return nc


def prep_inputs(x, h, W_ir, b_ir_lin, b_ir, W_h, b_hr, ncores=NCORES):
    """Host-side reshaping/casting/quantizing -> per-core in_maps."""
    import ml_dtypes

    E4 = ml_dtypes.float8_e4m3  # TRN float8e4 (IEEE-style, max 240)

    x = np.asarray(x, dtype=np.float32)
    h = np.asarray(h, dtype=np.float32)
    W_ir = np.asarray(W_ir, dtype=np.float32)
    W_h = np.asarray(W_h, dtype=np.float32)
    b_ir_lin = np.asarray(b_ir_lin, dtype=np.float32)
    b_ir = np.asarray(b_ir, dtype=np.float32)
    b_hr = np.asarray(b_hr, dtype=np.float32)

    bc = x.shape[0] // ncores

    # weights, gate-and-block reordered, contraction-dim-major.
    # fp8 paths pre-scaled by WSCALE so W ~ N(0,1.3) clears e4m3 min-normal;
    # wn carries the same scale in fp16 so both PSUM banks are uniform.
    Wr = W_ir[0:H].reshape(NB, BS, IN)
    Wz = W_ir[H : 2 * H].reshape(NB, BS, IN)
    Wn_ = W_ir[2 * H :].reshape(NB, BS, IN)
    wrz8 = (
        (np.concatenate([Wr, Wz], axis=1) * WSCALE)  # [NB, 512, IN]
        .transpose(2, 0, 1)
        .reshape(IN, NB * 2 * BS)
        .astype(E4)
    )
    wn = (Wn_ * WSCALE).transpose(2, 0, 1).reshape(IN, NB * BS).astype(np.float16)
    whrz8 = (
        (W_h[:, 0 : 2 * BS, :] * WSCALE)
        .transpose(2, 0, 1)
        .reshape(BS, NB * 2 * BS)
        .astype(E4)
    )
    whn8 = (
        (W_h[:, 2 * BS :, :] * WSCALE)
        .transpose(2, 0, 1)
        .reshape(BS, NB * BS)
        .astype(E4)
    )

    bx = b_ir_lin + b_ir
    bh = b_hr.reshape(NB, 3 * BS)
    brz = np.concatenate(
        [
            bx[0:H].reshape(NB, BS) + bh[:, 0:BS],
            bx[H : 2 * H].reshape(NB, BS) + bh[:, BS : 2 * BS],
        ],
        axis=1,
    ).reshape(1, NB * 2 * BS)
    bxn = bx[2 * H :].reshape(1, NB * BS).copy()
    bhn = bh[:, 2 * BS :].reshape(1, NB * BS).copy()
    has_bias = bool(np.any(brz) or np.any(bxn) or np.any(bhn))

    xT = np.ascontiguousarray(x.T)  # [IN, B]
    xT16 = xT.astype(np.float16)
    xT8 = xT.astype(E4)
    # h^T fp8, swizzled to [128, 16, B] (partition-major) so the device-side
    # bulk load is one contiguous descriptor per partition
    hT8 = h.T.astype(E4).reshape(16, P, B).transpose(1, 0, 2)
    hb1h = ((h + 1.0) * 0.5).astype(np.float16)  # [B, H]

    in_maps = []
    for c in range(ncores):
        csl = slice(c * bc, (c + 1) * bc)
        m = {
            "xt": np.ascontiguousarray(xT16[:, csl]),
            "x8": np.ascontiguousarray(xT8[:, csl]),
            "ht8": np.ascontiguousarray(hT8[:, :, csl]),
            "hb1h": np.ascontiguousarray(hb1h[csl]),
            "wrz8": wrz8,
            "wn": wn,
            "whrz8": whrz8,
            "whn8": whn8,
        }
        if has_bias:
            # biases carry the same *64 scale as the weights
            m["brz"] = (brz * WSCALE).astype(np.float32)
            m["bxn"] = (bxn * WSCALE).astype(np.float32)
            m["bhn"] = (bhn * WSCALE).astype(np.float32)
        in_maps.append(m)
    return in_maps, has_bias, bc


def kernel(x, h, W_ir, b_ir_lin, b_ir, W_h, b_hr):
    from concourse.bass_utils import run_bass_kernel_spmd

    in_maps, has_bias, bc = prep_inputs(x, h, W_ir, b_ir_lin, b_ir, W_h, b_hr)
    nc = build_nc(bc=bc, has_bias=has_bias)
    try:
        res = run_bass_kernel_spmd(nc, in_maps, list(range(NCORES)))
    except Exception:
        # transient NRT device errors have been observed once in ~10 runs;
        # a single retry reuses the compiled NEFF
        res = run_bass_kernel_spmd(nc, in_maps, list(range(NCORES)))
    return np.concatenate([res.results[c]["out"] for c in range(NCORES)], axis=0)
